# revision 1
# baseline (speedup 1.0000x reference)
"""AttnBlock (B=2, C=512, H=W=64) on 8 TRN2 NeuronCores.

Sharding: core c handles batch b=c//4 and query-quarter q=c%4 (1024 of 4096
query positions). Keys/values are computed redundantly per core from the
full batch image (group-norm needs all of it anyway). The key axis is
host-permuted per core so the core's query quarter occupies columns 0:1024
of its buffer — softmax/attention are permutation-invariant over keys, so
the same SPMD program works on every core with no dynamic indexing.

Attention is computed via S^T = k^T q (keys stationary): softmax runs
without max-subtraction (logits are ~N(0,1), exp is safe in fp32), the
exp(S^T) tiles feed the PV matmul directly as stationary operand, row sums
come from a ones-vector matmul, and 1/Z is folded into a final
per-partition scale.

Matmuls run in float32r (fp32 rounded to 11 mantissa bits, full PE rate).
Weights are pre-rounded on the host; on-device producers write f32r so the
PE consumes rounded values. The residual path stays exact fp32.
"""

import numpy as np

import concourse.bass as bass
import concourse.tile as tile
from concourse import bacc, mybir
from concourse.bass_utils import run_bass_kernel_spmd

F32 = mybir.dt.float32
F32R = mybir.dt.float32r

P = 128          # partitions
CT = 4           # channel tiles (C = 512 = 4*128)
C = 512
N = 4096         # H*W
NS = 8           # 512-wide column slices of N
NJT = 32         # 128-wide key tiles
NQ = 1024        # query columns per core
B = 2
HW = 64
NGROUPS = 32
GSIZE = C // NGROUPS  # 16 channels per group
EPS = 1e-5
SCL = float(C) ** -0.5
NCORES = 8

_cached = {}


def _round_f32r(a):
    """Round fp32 to 11 mantissa bits (RNE), keep fp32 container."""
    u = np.ascontiguousarray(a, dtype=np.float32).view(np.uint32)
    keep = np.uint32(0xFFFFF000)
    bias = np.uint32(0x800) - ((u >> np.uint32(12)) & np.uint32(1))
    return ((u + bias) & keep).view(np.float32)


def _ct_layout(v):
    """[C] -> [P, CT] with channel c at [c % 128, c // 128]."""
    return np.ascontiguousarray(v.reshape(CT, P).T, dtype=np.float32)


def _cmaj(a2d, ncols):
    """[C, ncols] -> [P, CT, ncols]."""
    return np.ascontiguousarray(
        a2d.reshape(CT, P, ncols).transpose(1, 0, 2), dtype=np.float32
    )


def _build_program():
    nc = bacc.Bacc("TRN2", target_bir_lowering=False, debug=False)

    X_d = nc.declare_dram_parameter("xin", [P, CT, N], F32R, isOutput=False)
    XQ_d = nc.declare_dram_parameter("xq", [P, CT, NQ], F32, isOutput=False)
    WQ_d = nc.declare_dram_parameter("wqt", [P, CT, C], F32R, isOutput=False)
    WK_d = nc.declare_dram_parameter("wkt", [P, CT, C], F32R, isOutput=False)
    WV_d = nc.declare_dram_parameter("wvt", [P, CT, C], F32R, isOutput=False)
    WP_d = nc.declare_dram_parameter("wpt", [P, CT, C], F32R, isOutput=False)
    BQ_d = nc.declare_dram_parameter("bq2", [P, CT], F32, isOutput=False)
    BK_d = nc.declare_dram_parameter("bk2", [P, CT], F32, isOutput=False)
    BPE_d = nc.declare_dram_parameter("bpe", [P, CT], F32, isOutput=False)
    GAM_d = nc.declare_dram_parameter("gam", [P, CT], F32, isOutput=False)
    BET_d = nc.declare_dram_parameter("bet", [P, CT], F32, isOutput=False)
    G_d = nc.declare_dram_parameter("gmat", [P, CT, NGROUPS], F32, isOutput=False)
    E_d = nc.declare_dram_parameter("emat", [NGROUPS, CT, P], F32, isOutput=False)
    ID_d = nc.declare_dram_parameter("ident", [P, P], F32, isOutput=False)
    ONE_d = nc.declare_dram_parameter("ones1", [P, 1], F32, isOutput=False)
    OF_d = nc.declare_dram_parameter("onef", [1, 1], F32, isOutput=False)
    OUT_d = nc.declare_dram_parameter("out", [P, CT, NQ], F32, isOutput=True)

    with tile.TileContext(nc) as tc:
        with (
            tc.tile_pool(name="big", bufs=1) as big,
            tc.tile_pool(name="consts", bufs=1) as consts,
            tc.tile_pool(name="stat", bufs=1) as stat,
        ):
            X = big.tile([P, CT, N], F32R)
            VT = big.tile([P, NJT, C], F32R)
            QO = big.tile([P, CT, NQ], F32R)
            SPARE = big.tile([P, CT, 512], F32R)

            wp = consts.tile([P, CT, C], F32R)
            bpe_sb = consts.tile([P, CT], F32)
            bq_sb = consts.tile([P, CT], F32)
            bk_sb = consts.tile([P, CT], F32)
            gam_sb = consts.tile([P, CT], F32)
            bet_sb = consts.tile([P, CT], F32)
            gmat = consts.tile([P, CT, NGROUPS], F32)
            emat = consts.tile([NGROUPS, CT, P], F32)
            ident = consts.tile([P, P], F32)
            ones1 = consts.tile([P, 1], F32)
            onef = consts.tile([1, 1], F32)

            nc.sync.dma_start(out=ident, in_=ID_d[:])
            for s in range(NS):
                sl = slice(s * 512, (s + 1) * 512)
                nc.sync.dma_start(out=X[:, :, sl], in_=X_d[:, :, sl])
            nc.sync.dma_start(out=gmat, in_=G_d[:])
            nc.sync.dma_start(out=emat, in_=E_d[:])
            nc.sync.dma_start(out=gam_sb, in_=GAM_d[:])
            nc.sync.dma_start(out=bet_sb, in_=BET_d[:])
            nc.sync.dma_start(out=bq_sb, in_=BQ_d[:])
            nc.sync.dma_start(out=bk_sb, in_=BK_d[:])
            nc.sync.dma_start(out=ones1, in_=ONE_d[:])
            nc.sync.dma_start(out=onef, in_=OF_d[:])

            # ---------------- Phase 1: group-norm statistics ----------------
            bnst = stat.tile([P, CT, NS, 6], F32)
            for s in range(NS):
                for t in range(CT):
                    nc.vector.bn_stats(
                        out=bnst[:, t, s, :],
                        in_=X[:, t, s * 512 : (s + 1) * 512].bitcast(F32),
                    )
            mex = stat.tile([P, CT, 2], F32)
            for t in range(CT):
                nc.vector.bn_aggr(out=mex[:, t, :], in_=bnst[:, t, :, :])
            # mexp[...,0] = mean, mexp[...,1] = E[x^2] = var + mean^2
            mexp = stat.tile([P, CT, 2], F32)
            nc.vector.tensor_copy(out=mexp[:, :, 0], in_=mex[:, :, 0])
            nc.vector.tensor_tensor(
                out=mexp[:, :, 1], in0=mex[:, :, 0], in1=mex[:, :, 0],
                op=mybir.AluOpType.mult,
            )
            nc.vector.tensor_add(
                out=mexp[:, :, 1], in0=mexp[:, :, 1], in1=mex[:, :, 1]
            )

            scale_c = stat.tile([P, CT], F32)
            shift_c = stat.tile([P, CT], F32)
            with tc.tile_pool(name="psum_p1", bufs=1, space="PSUM") as p1:
                gs_ps = p1.tile([NGROUPS, 2], F32, tag="gs")
                for t in range(CT):
                    nc.tensor.matmul(
                        gs_ps, gmat[:, t, :], mexp[:, t, :],
                        start=(t == 0), stop=(t == CT - 1),
                    )
                gsb = stat.tile([NGROUPS, 2], F32)
                nc.vector.tensor_copy(out=gsb, in_=gs_ps)
                gmr = stat.tile([NGROUPS, 2], F32)
                gtmp = stat.tile([NGROUPS, 2], F32)
                nc.scalar.mul(out=gmr[:, 0:1], in_=gsb[:, 0:1], mul=1.0 / GSIZE)
                nc.scalar.mul(out=gtmp[:, 0:1], in_=gsb[:, 1:2], mul=1.0 / GSIZE)
                nc.vector.tensor_tensor(
                    out=gtmp[:, 1:2], in0=gmr[:, 0:1], in1=gmr[:, 0:1],
                    op=mybir.AluOpType.mult,
                )
                nc.vector.tensor_sub(
                    out=gtmp[:, 0:1], in0=gtmp[:, 0:1], in1=gtmp[:, 1:2]
                )
                eps_sb = stat.tile([NGROUPS, 1], F32)
                nc.vector.memset(eps_sb, EPS)
                nc.scalar.activation(
                    out=gtmp[:, 0:1], in_=gtmp[:, 0:1],
                    func=mybir.ActivationFunctionType.Sqrt, bias=eps_sb,
                )
                nc.vector.reciprocal(out=gmr[:, 1:2], in_=gtmp[:, 0:1])
                mc = stat.tile([P, CT, 2], F32)
                for t in range(CT):
                    ms_ps = p1.tile([P, 2], F32, tag="ms")
                    nc.tensor.matmul(ms_ps, emat[:, t, :], gmr, start=True, stop=True)
                    nc.vector.tensor_copy(out=mc[:, t, :], in_=ms_ps)
                nc.vector.tensor_tensor(
                    out=scale_c, in0=mc[:, :, 1], in1=gam_sb, op=mybir.AluOpType.mult
                )
                nc.vector.tensor_tensor(
                    out=shift_c, in0=mc[:, :, 0], in1=scale_c, op=mybir.AluOpType.mult
                )
                nc.vector.tensor_sub(out=shift_c, in0=bet_sb, in1=shift_c)

            # ---------------- Phase 2: normalize + q/k/vT projections -------
            def norm_slice(s):
                sl = slice(s * 512, (s + 1) * 512)
                for t in range(CT):
                    nc.vector.tensor_scalar(
                        out=X[:, t, sl],
                        in0=X[:, t, sl].bitcast(F32),
                        scalar1=scale_c[:, t : t + 1],
                        scalar2=shift_c[:, t : t + 1],
                        op0=mybir.AluOpType.mult,
                        op1=mybir.AluOpType.add,
                    )

            with (
                tc.tile_pool(name="wqkv", bufs=1) as wpool,
                tc.tile_pool(name="psum2", bufs=1, space="PSUM") as psum2,
            ):
                wq = wpool.tile([P, CT, C], F32R)
                wk = wpool.tile([P, CT, C], F32R)
                wv = wpool.tile([P, CT, C], F32R)
                nc.sync.dma_start(out=wq, in_=WQ_d[:])
                nc.sync.dma_start(out=wk, in_=WK_d[:])
                nc.sync.dma_start(out=wv, in_=WV_d[:])
                nc.sync.dma_start(out=wp, in_=WP_d[:])
                nc.sync.dma_start(out=bpe_sb, in_=BPE_d[:])

                norm_slice(0)
                for s in range(NS):
                    if s + 1 < NS:
                        norm_slice(s + 1)
                    sl = slice(s * 512, (s + 1) * 512)
                    if s < 2:
                        for ct in range(CT):
                            qp = psum2.tile([P, 512], F32, tag="acc", bufs=3)
                            for kt in range(CT):
                                nc.tensor.matmul(
                                    qp,
                                    wq[:, kt, ct * P : (ct + 1) * P],
                                    X[:, kt, sl],
                                    start=(kt == 0), stop=(kt == CT - 1),
                                )
                            nc.scalar.activation(
                                out=QO[:, ct, s * 512 : (s + 1) * 512], in_=qp,
                                func=mybir.ActivationFunctionType.Identity,
                                bias=bq_sb[:, ct : ct + 1],
                            )
                    for jt in range(CT):
                        vp = psum2.tile([P, 512], F32, tag="acc", bufs=3)
                        jcol = slice(s * 512 + jt * P, s * 512 + (jt + 1) * P)
                        for kt in range(CT):
                            nc.tensor.matmul(
                                vp, X[:, kt, jcol], wv[:, kt, :],
                                start=(kt == 0), stop=(kt == CT - 1),
                            )
                        nc.vector.tensor_copy(out=VT[:, s * 4 + jt, :], in_=vp)
                    # k overwrites the previous (dead) slice region; k(0)->SPARE
                    for ct in range(CT):
                        kp = psum2.tile([P, 512], F32, tag="acc", bufs=3)
                        for kt in range(CT):
                            nc.tensor.matmul(
                                kp,
                                wk[:, kt, ct * P : (ct + 1) * P],
                                X[:, kt, sl],
                                start=(kt == 0), stop=(kt == CT - 1),
                            )
                        if s == 0:
                            kdst = SPARE[:, ct, :]
                        else:
                            kdst = X[:, ct, (s - 1) * 512 : s * 512]
                        nc.scalar.activation(
                            out=kdst, in_=kp,
                            func=mybir.ActivationFunctionType.Identity,
                            bias=bk_sb[:, ct : ct + 1],
                        )

            # ---------------- Phase 3: attention (S^T route) -----------------
            def key_block(jt, kt):
                """[128 c, 128 j] block of keys for global key tile jt."""
                js, sub = jt // 4, jt % 4
                if js == 0:
                    return SPARE[:, kt, sub * P : (sub + 1) * P]
                base = (js - 1) * 512 + sub * P
                return X[:, kt, base : base + P]

            with (
                tc.tile_pool(name="psum3", bufs=1, space="PSUM") as psum3,
                tc.tile_pool(name="pwork", bufs=1) as pwork,
            ):
                deferred = []

                def pop_deferred():
                    if deferred:
                        deferred.pop(0)()

                def st_group(isl, jt):
                    """S^T matmuls + exp for key tile jt against i-slice isl."""
                    s_ps = psum3.tile([P, 512], F32, tag="s", bufs=2)
                    isl_sl = slice(isl * 512, (isl + 1) * 512)
                    for kt in range(CT):
                        nc.tensor.matmul(
                            s_ps,
                            key_block(jt, kt),
                            QO[:, kt, isl_sl],
                            start=(kt == 0), stop=(kt == CT - 1),
                        )
                    pt = pwork.tile([P, 512], F32R, tag="p", bufs=4)
                    nc.scalar.activation(
                        out=pt, in_=s_ps,
                        func=mybir.ActivationFunctionType.Exp, scale=SCL,
                    )
                    return pt

                def emit_znorm(isl, zsum, u_list):
                    """Normalize u blocks by 1/Z immediately (frees u banks)."""
                    z_ps = psum3.tile([1, 512], F32, tag="t", bufs=2)
                    nc.tensor.matmul(z_ps, ones1, zsum, start=True, stop=True)
                    zrow = pwork.tile([1, 512], F32, tag="zrow", bufs=2)
                    nc.vector.tensor_copy(out=zrow, in_=z_ps)
                    nc.vector.reciprocal(out=zrow, in_=zrow)
                    osbs = []
                    for ib in range(4):
                        zx_ps = psum3.tile([P, 1], F32, tag="t", bufs=2)
                        nc.tensor.matmul(
                            zx_ps, zrow[:, ib * P : (ib + 1) * P], onef,
                            start=True, stop=True,
                        )
                        zinv = pwork.tile([P, 1], F32, tag="zinv", bufs=2)
                        nc.vector.tensor_copy(out=zinv, in_=zx_ps)
                        osb = pwork.tile([P, C], F32R, tag="osb", bufs=4)
                        nc.vector.tensor_scalar_mul(
                            out=osb, in0=u_list[ib], scalar1=zinv
                        )
                        osbs.append(osb)
                    return osbs

                def otr_closures(isl, osbs):
                    """Deferred: transpose normalized O^T blocks into QO."""
                    ops = []
                    for ib in range(4):
                        for ct in range(CT):
                            def otr(ib=ib, ct=ct):
                                t_ps = psum3.tile([P, P], F32, tag="t", bufs=2)
                                nc.tensor.transpose(
                                    t_ps,
                                    osbs[ib][:, ct * P : (ct + 1) * P].bitcast(F32),
                                    ident,
                                )
                                nc.vector.tensor_copy(
                                    out=QO[:, ct, isl * 512 + ib * P : isl * 512 + (ib + 1) * P],
                                    in_=t_ps,
                                )

                            ops.append(otr)
                    return ops

                def proj_group(h, ct):
                    """Projection + bias + residual + store for one 128x512
                    output block. Requires O (QO cols of i-slice h) final."""
                    sl = slice(h * 512, (h + 1) * 512)
                    pr = psum3.tile([P, 512], F32, tag="s", bufs=2)
                    for kt in range(CT):
                        nc.tensor.matmul(
                            pr,
                            wp[:, kt, ct * P : (ct + 1) * P],
                            QO[:, kt, sl],
                            start=(kt == 0), stop=(kt == CT - 1),
                        )
                    xqt = pwork.tile([P, 512], F32, tag="xqt", bufs=3)
                    nc.sync.dma_start(out=xqt, in_=XQ_d[:, ct, sl])
                    ost = pwork.tile([P, 512], F32, tag="ost", bufs=3)
                    nc.vector.scalar_tensor_tensor(
                        out=ost, in0=pr, scalar=bpe_sb[:, ct : ct + 1],
                        in1=xqt, op0=mybir.AluOpType.add,
                        op1=mybir.AluOpType.add,
                    )
                    nc.sync.dma_start(out=OUT_d[:, ct, sl], in_=ost)

                for isl in range(2):
                    zsum = pwork.tile([P, 512], F32, tag="zsum", bufs=2)
                    u_list = [
                        psum3.tile([P, C], F32, tag=f"u{ib}", bufs=1, name=f"u{ib}")
                        for ib in range(4)
                    ]
                    cur_pt = st_group(isl, 0)
                    for jt in range(NJT):
                        if jt + 1 < NJT:
                            nxt_pt = st_group(isl, jt + 1)
                        if jt == 0:
                            nc.vector.tensor_copy(out=zsum, in_=cur_pt.bitcast(F32))
                        else:
                            nc.vector.tensor_add(
                                out=zsum, in0=zsum, in1=cur_pt.bitcast(F32)
                            )
                        for ib in range(4):
                            nc.tensor.matmul(
                                u_list[ib],
                                cur_pt[:, ib * P : (ib + 1) * P],
                                VT[:, jt, :],
                                start=(jt == 0), stop=(jt == NJT - 1),
                            )
                        pop_deferred()
                        # i-slice 0's O is final once its 16 transposes popped
                        # (by jt=15 of isl 1) — run the h=0 projection here.
                        if isl == 1 and jt >= 17 and (jt - 17) % 4 == 0:
                            proj_group(0, (jt - 17) // 4)
                        if jt + 1 < NJT:
                            cur_pt = nxt_pt
                    osbs = emit_znorm(isl, zsum, u_list)
                    deferred.extend(otr_closures(isl, osbs))

                # ---------------- Phase 4: remaining projection (h=1) --------
                # i-slice 1's O-transposes must fully drain before h=1 emits
                # (emission order defines the dependency graph).
                while deferred:
                    pop_deferred()
                for ct in range(CT):
                    proj_group(1, ct)

    nc.compile()
    return nc


def _get_nc():
    if "nc" not in _cached:
        _cached["nc"] = _build_program()
    return _cached["nc"]


def _make_in_maps(x, norm_gamma, norm_beta, wq, bq, wk, bk, wv, bv, wp, bp):
    gm = np.zeros((P, CT, NGROUPS), np.float32)
    em = np.zeros((NGROUPS, CT, P), np.float32)
    for t in range(CT):
        for p in range(P):
            g = (t * P + p) // GSIZE
            gm[p, t, g] = 1.0
            em[g, t, p] = 1.0

    common = {
        "wqt": _round_f32r(_cmaj(np.asarray(wq).T, C)),
        "wkt": _round_f32r(_cmaj(np.asarray(wk).T, C)),
        "wvt": _round_f32r(_cmaj(np.asarray(wv).T, C)),
        "wpt": _round_f32r(_cmaj(np.asarray(wp).T, C)),
        "bq2": _ct_layout(np.asarray(bq)),
        "bk2": _ct_layout(np.asarray(bk)),
        "bpe": _ct_layout(np.asarray(bp) + np.asarray(wp) @ np.asarray(bv)),
        "gam": _ct_layout(np.asarray(norm_gamma)),
        "bet": _ct_layout(np.asarray(norm_beta)),
        "gmat": gm,
        "emat": em,
        "ident": np.eye(P, dtype=np.float32),
        "ones1": np.ones((P, 1), np.float32),  # fp32 (exact) reducer vector
        "onef": np.ones((1, 1), np.float32),
    }

    in_maps = []
    for c in range(NCORES):
        b, qi = c // 4, c % 4
        xb = np.asarray(x[b], dtype=np.float32).reshape(C, N)
        xp = np.concatenate([xb[:, qi * NQ :], xb[:, : qi * NQ]], axis=1)
        m = dict(common)
        m["xin"] = _round_f32r(_cmaj(xp, N))
        m["xq"] = _cmaj(xb[:, qi * NQ : (qi + 1) * NQ], NQ)
        in_maps.append(m)
    return in_maps


def _assemble(results):
    out = np.empty((B, C, N), np.float32)
    for c in range(NCORES):
        b, qi = c // 4, c % 4
        r = results[c]["out"]  # [P, CT, NQ]
        out[b, :, qi * NQ : (qi + 1) * NQ] = (
            r.transpose(1, 0, 2).reshape(C, NQ)
        )
    return out.reshape(B, C, HW, HW)


def _run(inputs, trace=False, trace_kwargs=None):
    nc = _get_nc()
    in_maps = _make_in_maps(**inputs)
    res = run_bass_kernel_spmd(
        nc, in_maps, list(range(NCORES)), trace=trace,
        **(trace_kwargs or {}),
    )
    return res


def kernel(**inputs):
    res = _run(inputs)
    return _assemble(res.results)



# revision 6
# speedup vs baseline: 1.2565x; 1.2565x over previous
"""AttnBlock (B=2, C=512, H=W=64) on 8 TRN2 NeuronCores.

Sharding: core c handles batch b=c//4 and query-quarter q=c%4 (1024 of 4096
query positions). Keys/values are computed redundantly per core from the
full batch image (group-norm needs all of it anyway). The key axis is
host-permuted per core so the core's query quarter occupies columns 0:1024
of its buffer — softmax/attention are permutation-invariant over keys, so
the same SPMD program works on every core with no dynamic indexing.

All matmuls run in fp8 e4m3 DoubleRow mode (two 128-deep contraction
subtiles per instruction, 2x PE rate). x is shipped bf16; group-norm
statistics run on bf16 via bn_stats. Softmax runs without max-subtraction
(logits ~N(0,1)); exp writes fp8 probability pair-tiles that feed the PE
directly: an all-ones stationary produces Z replicated across partitions
(so 1/Z is one full-width reciprocal), and U = V P^T is accumulated
directly in [c, i] layout so no transposes are needed — U is normalized
by 1/Z during the PSUM->fp8 cast and fed straight to the output
projection. The residual path stays exact fp32.
"""

import numpy as np
import ml_dtypes

import concourse.bass as bass
import concourse.tile as tile
from concourse import bacc, mybir
from concourse.bass_utils import run_bass_kernel_spmd

F32 = mybir.dt.float32
BF16 = mybir.dt.bfloat16
F8 = mybir.dt.float8e4
DR = mybir.MatmulPerfMode.DoubleRow
E4 = ml_dtypes.float8_e4m3fn

P = 128          # partitions
CT = 4           # channel tiles (C = 512 = 4*128)
C = 512
N = 4096         # H*W
NS = 8           # 512-wide column slices of N
NJT = 32         # 128-wide key tiles
NPAIR = 16       # key-tile pairs (DoubleRow contraction)
NQ = 1024        # query columns per core
B = 2
HW = 64
NGROUPS = 32
GSIZE = C // NGROUPS  # 16 channels per group
EPS = 1e-5
SCL = float(C) ** -0.5
NCORES = 8

_cached = {}


def _ct_layout(v):
    """[C] -> [P, CT] with channel c at [c % 128, c // 128]."""
    return np.ascontiguousarray(v.reshape(CT, P).T, dtype=np.float32)


def _cmaj(a2d, ncols, dtype=np.float32):
    """[C, ncols] -> [P, CT, ncols]."""
    return np.ascontiguousarray(
        a2d.reshape(CT, P, ncols).transpose(1, 0, 2).astype(dtype)
    )


def _build_program():
    nc = bacc.Bacc("TRN2", target_bir_lowering=False, debug=False)

    X_d = nc.declare_dram_parameter("xin", [P, CT, N], BF16, isOutput=False)
    XQ_d = nc.declare_dram_parameter("xq", [P, CT, NQ], F32, isOutput=False)
    WQ_d = nc.declare_dram_parameter("wqt", [P, CT, C], F8, isOutput=False)
    WK_d = nc.declare_dram_parameter("wkt", [P, CT, C], F8, isOutput=False)
    WV_d = nc.declare_dram_parameter("wvt", [P, CT, C], F8, isOutput=False)
    WP_d = nc.declare_dram_parameter("wpt", [P, CT, C], F8, isOutput=False)
    BQ_d = nc.declare_dram_parameter("bq2", [P, CT], F32, isOutput=False)
    BK_d = nc.declare_dram_parameter("bk2", [P, CT], F32, isOutput=False)
    BPE_d = nc.declare_dram_parameter("bpe", [P, CT], F32, isOutput=False)
    GAM_d = nc.declare_dram_parameter("gam", [P, CT], F32, isOutput=False)
    BET_d = nc.declare_dram_parameter("bet", [P, CT], F32, isOutput=False)
    G_d = nc.declare_dram_parameter("gmat", [P, CT, NGROUPS], F32, isOutput=False)
    E_d = nc.declare_dram_parameter("emat", [NGROUPS, CT, P], F32, isOutput=False)
    ONE_d = nc.declare_dram_parameter("ones8", [P, 2, P], F8, isOutput=False)
    OUT_d = nc.declare_dram_parameter("out", [P, CT, NQ], F32, isOutput=True)

    with tile.TileContext(nc) as tc:
        with (
            tc.tile_pool(name="big", bufs=1) as big,
            tc.tile_pool(name="consts", bufs=1) as consts,
            tc.tile_pool(name="stat", bufs=1) as stat,
        ):
            XB = big.tile([P, CT, N], BF16)
            H8 = big.tile([P, CT, N], F8)
            K8 = big.tile([P, CT, N], F8)
            V8 = big.tile([P, NJT, C], F8)
            Q8 = big.tile([P, CT, NQ], F8)
            U8 = big.tile([P, CT, NQ], F8)

            wq = consts.tile([P, CT, C], F8)
            wk = consts.tile([P, CT, C], F8)
            wv = consts.tile([P, CT, C], F8)
            wp = consts.tile([P, CT, C], F8)
            ones8 = consts.tile([P, 2, P], F8)
            bpe_sb = consts.tile([P, CT], F32)
            bq_sb = consts.tile([P, CT], F32)
            bk_sb = consts.tile([P, CT], F32)
            gam_sb = consts.tile([P, CT], F32)
            bet_sb = consts.tile([P, CT], F32)
            gmat = consts.tile([P, CT, NGROUPS], F32)
            emat = consts.tile([NGROUPS, CT, P], F32)
            # logit shift: exp(SCL*s - 2) keeps p under fp8e4m3 max (448)
            # while staying softmax-invariant (cancels in U/Z)
            neg2 = consts.tile([P, 1], F32)
            nc.vector.memset(neg2, -2.0)

            for s in range(NS):
                sl = slice(s * 512, (s + 1) * 512)
                nc.sync.dma_start(out=XB[:, :, sl], in_=X_d[:, :, sl])
            nc.sync.dma_start(out=ones8, in_=ONE_d[:])
            nc.sync.dma_start(out=gmat, in_=G_d[:])
            nc.sync.dma_start(out=emat, in_=E_d[:])
            nc.sync.dma_start(out=gam_sb, in_=GAM_d[:])
            nc.sync.dma_start(out=bet_sb, in_=BET_d[:])
            nc.sync.dma_start(out=bq_sb, in_=BQ_d[:])
            nc.sync.dma_start(out=bk_sb, in_=BK_d[:])
            nc.sync.dma_start(out=wq, in_=WQ_d[:])
            nc.sync.dma_start(out=wk, in_=WK_d[:])
            nc.sync.dma_start(out=wv, in_=WV_d[:])
            nc.sync.dma_start(out=wp, in_=WP_d[:])
            nc.sync.dma_start(out=bpe_sb, in_=BPE_d[:])

            # ---------------- Phase 1: group-norm statistics ----------------
            bnst = stat.tile([P, CT, NS, 6], F32)
            for s in range(NS):
                for t in range(CT):
                    nc.vector.bn_stats(
                        out=bnst[:, t, s, :],
                        in_=XB[:, t, s * 512 : (s + 1) * 512],
                    )
            mex = stat.tile([P, CT, 2], F32)
            for t in range(CT):
                nc.vector.bn_aggr(out=mex[:, t, :], in_=bnst[:, t, :, :])
            # mexp[...,0] = mean, mexp[...,1] = E[x^2] = var + mean^2
            mexp = stat.tile([P, CT, 2], F32)
            nc.vector.tensor_copy(out=mexp[:, :, 0], in_=mex[:, :, 0])
            nc.vector.tensor_tensor(
                out=mexp[:, :, 1], in0=mex[:, :, 0], in1=mex[:, :, 0],
                op=mybir.AluOpType.mult,
            )
            nc.vector.tensor_add(
                out=mexp[:, :, 1], in0=mexp[:, :, 1], in1=mex[:, :, 1]
            )

            scale_c = stat.tile([P, CT], F32)
            shift_c = stat.tile([P, CT], F32)
            with tc.tile_pool(name="psum_p1", bufs=1, space="PSUM") as p1:
                gs_ps = p1.tile([NGROUPS, 2], F32, tag="gs")
                for t in range(CT):
                    nc.tensor.matmul(
                        gs_ps, gmat[:, t, :], mexp[:, t, :],
                        start=(t == 0), stop=(t == CT - 1),
                    )
                gsb = stat.tile([NGROUPS, 2], F32)
                nc.vector.tensor_copy(out=gsb, in_=gs_ps)
                gmr = stat.tile([NGROUPS, 2], F32)
                gtmp = stat.tile([NGROUPS, 2], F32)
                nc.scalar.mul(out=gmr[:, 0:1], in_=gsb[:, 0:1], mul=1.0 / GSIZE)
                nc.scalar.mul(out=gtmp[:, 0:1], in_=gsb[:, 1:2], mul=1.0 / GSIZE)
                nc.vector.tensor_tensor(
                    out=gtmp[:, 1:2], in0=gmr[:, 0:1], in1=gmr[:, 0:1],
                    op=mybir.AluOpType.mult,
                )
                nc.vector.tensor_sub(
                    out=gtmp[:, 0:1], in0=gtmp[:, 0:1], in1=gtmp[:, 1:2]
                )
                eps_sb = stat.tile([NGROUPS, 1], F32)
                nc.vector.memset(eps_sb, EPS)
                nc.scalar.activation(
                    out=gtmp[:, 0:1], in_=gtmp[:, 0:1],
                    func=mybir.ActivationFunctionType.Sqrt, bias=eps_sb,
                )
                nc.vector.reciprocal(out=gmr[:, 1:2], in_=gtmp[:, 0:1])
                mc = stat.tile([P, CT, 2], F32)
                for t in range(CT):
                    ms_ps = p1.tile([P, 2], F32, tag="ms")
                    nc.tensor.matmul(ms_ps, emat[:, t, :], gmr, start=True, stop=True)
                    nc.vector.tensor_copy(out=mc[:, t, :], in_=ms_ps)
                nc.vector.tensor_tensor(
                    out=scale_c, in0=mc[:, :, 1], in1=gam_sb, op=mybir.AluOpType.mult
                )
                nc.vector.tensor_tensor(
                    out=shift_c, in0=mc[:, :, 0], in1=scale_c, op=mybir.AluOpType.mult
                )
                nc.vector.tensor_sub(out=shift_c, in0=bet_sb, in1=shift_c)

            # ---------------- Phase 2: normalize + q/k/v projections --------
            def norm_slice(s):
                sl = slice(s * 512, (s + 1) * 512)
                for t in range(CT):
                    nc.gpsimd.tensor_scalar(
                        out=H8[:, t, sl],
                        in0=XB[:, t, sl],
                        scalar1=scale_c[:, t : t + 1],
                        scalar2=shift_c[:, t : t + 1],
                        op0=mybir.AluOpType.mult,
                        op1=mybir.AluOpType.add,
                    )

            with tc.tile_pool(name="psum2", bufs=1, space="PSUM") as psum2:
                norm_slice(0)
                for s in range(NS):
                    if s + 1 < NS:
                        norm_slice(s + 1)
                    sl = slice(s * 512, (s + 1) * 512)
                    if s < 2:
                        for ct in range(CT):
                            qp = psum2.tile([P, 512], F32, tag="acc", bufs=3)
                            for m in range(2):
                                nc.tensor.matmul(
                                    qp,
                                    wq[:, 2 * m : 2 * m + 2, ct * P : (ct + 1) * P],
                                    H8[:, 2 * m : 2 * m + 2, sl],
                                    start=(m == 0), stop=(m == 1), perf_mode=DR,
                                )
                            nc.vector.tensor_scalar_add(
                                out=Q8[:, ct, sl],
                                in0=qp,
                                scalar1=bq_sb[:, ct : ct + 1],
                            )
                    for jt in range(CT):
                        vp = psum2.tile([P, 512], F32, tag="acc", bufs=3)
                        jcol = slice(s * 512 + jt * P, s * 512 + (jt + 1) * P)
                        for m in range(2):
                            nc.tensor.matmul(
                                vp,
                                H8[:, 2 * m : 2 * m + 2, jcol],
                                wv[:, 2 * m : 2 * m + 2, :],
                                start=(m == 0), stop=(m == 1), perf_mode=DR,
                            )
                        nc.scalar.activation(
                            out=V8[:, s * 4 + jt, :], in_=vp,
                            func=mybir.ActivationFunctionType.Identity,
                        )
                    for ct in range(CT):
                        kp = psum2.tile([P, 512], F32, tag="acc", bufs=3)
                        for m in range(2):
                            nc.tensor.matmul(
                                kp,
                                wk[:, 2 * m : 2 * m + 2, ct * P : (ct + 1) * P],
                                H8[:, 2 * m : 2 * m + 2, sl],
                                start=(m == 0), stop=(m == 1), perf_mode=DR,
                            )
                        nc.vector.tensor_scalar_add(
                            out=K8[:, ct, sl],
                            in0=kp,
                            scalar1=bk_sb[:, ct : ct + 1],
                        )

            # ---------------- Phase 3: attention --------------------------
            with (
                tc.tile_pool(name="psum3", bufs=1, space="PSUM") as psum3,
                tc.tile_pool(name="pwork", bufs=1) as pwork,
            ):
                proj_jobs = []

                def pop_proj():
                    if proj_jobs:
                        proj_jobs.pop(0)()

                def st_exp(isl, jt, pt):
                    """S^T matmuls + exp into pair-tile half jt%2."""
                    s_ps = psum3.tile([P, 512], F32, tag="s", bufs=2)
                    isl_sl = slice(isl * 512, (isl + 1) * 512)
                    jb = slice(jt * P, (jt + 1) * P)
                    for m in range(2):
                        nc.tensor.matmul(
                            s_ps,
                            K8[:, 2 * m : 2 * m + 2, jb],
                            Q8[:, 2 * m : 2 * m + 2, isl_sl],
                            start=(m == 0), stop=(m == 1), perf_mode=DR,
                        )
                    nc.scalar.activation(
                        out=pt[:, jt % 2, :], in_=s_ps,
                        func=mybir.ActivationFunctionType.Exp, scale=SCL,
                        bias=neg2,
                    )

                def proj_group(h, ct):
                    """Projection + bias + residual + store for one 128x512
                    output block. Requires U8 cols of i-slice h final."""
                    sl = slice(h * 512, (h + 1) * 512)
                    pr = psum3.tile([P, 512], F32, tag="s", bufs=2)
                    for m in range(2):
                        nc.tensor.matmul(
                            pr,
                            wp[:, 2 * m : 2 * m + 2, ct * P : (ct + 1) * P],
                            U8[:, 2 * m : 2 * m + 2, sl],
                            start=(m == 0), stop=(m == 1), perf_mode=DR,
                        )
                    xqt = pwork.tile([P, 512], F32, tag="xqt", bufs=3)
                    nc.sync.dma_start(out=xqt, in_=XQ_d[:, ct, sl])
                    ost = pwork.tile([P, 512], F32, tag="ost", bufs=3)
                    nc.vector.scalar_tensor_tensor(
                        out=ost, in0=pr, scalar=bpe_sb[:, ct : ct + 1],
                        in1=xqt, op0=mybir.AluOpType.add,
                        op1=mybir.AluOpType.add,
                    )
                    nc.sync.dma_start(out=OUT_d[:, ct, sl], in_=ost)

                for isl in range(2):
                    u_ps = [
                        psum3.tile([P, 512], F32, tag=f"u{cc}", bufs=1,
                                   name=f"u{cc}")
                        for cc in range(CT)
                    ]
                    z_ps = psum3.tile([P, 512], F32, tag="z", bufs=1, name="z")
                    for pair in range(NPAIR):
                        pt = pwork.tile([P, 2, 512], F8, tag="pt", bufs=3)
                        st_exp(isl, 2 * pair, pt)
                        st_exp(isl, 2 * pair + 1, pt)
                        nc.tensor.matmul(
                            z_ps, ones8, pt,
                            start=(pair == 0), stop=(pair == NPAIR - 1),
                            perf_mode=DR,
                        )
                        for cc in range(CT):
                            nc.tensor.matmul(
                                u_ps[cc],
                                V8[:, 2 * pair : 2 * pair + 2,
                                   cc * P : (cc + 1) * P],
                                pt,
                                start=(pair == 0), stop=(pair == NPAIR - 1),
                                perf_mode=DR,
                            )
                        # interleave i-slice 0's output projection into
                        # i-slice 1's key loop so the PE never stalls
                        if isl == 1 and pair >= 2 and (pair - 2) % 3 == 0:
                            pop_proj()
                    zb = pwork.tile([P, 512], F32, tag="zb", bufs=2)
                    nc.vector.reciprocal(out=zb, in_=z_ps)
                    isl_sl = slice(isl * 512, (isl + 1) * 512)
                    for cc in range(CT):
                        nc.vector.tensor_tensor(
                            out=U8[:, cc, isl_sl], in0=u_ps[cc], in1=zb,
                            op=mybir.AluOpType.mult,
                        )
                    for ct in range(CT):
                        proj_jobs.append(lambda h=isl, ct=ct: proj_group(h, ct))

                while proj_jobs:
                    pop_proj()

    nc.compile()
    return nc


def _get_nc():
    if "nc" not in _cached:
        _cached["nc"] = _build_program()
    return _cached["nc"]


def _make_in_maps(x, norm_gamma, norm_beta, wq, bq, wk, bk, wv, bv, wp, bp):
    gm = np.zeros((P, CT, NGROUPS), np.float32)
    em = np.zeros((NGROUPS, CT, P), np.float32)
    for t in range(CT):
        for p in range(P):
            g = (t * P + p) // GSIZE
            gm[p, t, g] = 1.0
            em[g, t, p] = 1.0

    common = {
        "wqt": _cmaj(np.asarray(wq).T, C, E4),
        "wkt": _cmaj(np.asarray(wk).T, C, E4),
        "wvt": _cmaj(np.asarray(wv).T, C, E4),
        "wpt": _cmaj(np.asarray(wp).T, C, E4),
        "bq2": _ct_layout(np.asarray(bq)),
        "bk2": _ct_layout(np.asarray(bk)),
        "bpe": _ct_layout(np.asarray(bp) + np.asarray(wp) @ np.asarray(bv)),
        "gam": _ct_layout(np.asarray(norm_gamma)),
        "bet": _ct_layout(np.asarray(norm_beta)),
        "gmat": gm,
        "emat": em,
        "ones8": np.ones((P, 2, P), dtype=E4),
    }

    in_maps = []
    for c in range(NCORES):
        b, qi = c // 4, c % 4
        xb = np.asarray(x[b], dtype=np.float32).reshape(C, N)
        xp = np.concatenate([xb[:, qi * NQ :], xb[:, : qi * NQ]], axis=1)
        m = dict(common)
        m["xin"] = _cmaj(xp, N, ml_dtypes.bfloat16)
        m["xq"] = _cmaj(xb[:, qi * NQ : (qi + 1) * NQ], NQ)
        in_maps.append(m)
    return in_maps


def _assemble(results):
    out = np.empty((B, C, N), np.float32)
    for c in range(NCORES):
        b, qi = c // 4, c % 4
        r = results[c]["out"]  # [P, CT, NQ]
        out[b, :, qi * NQ : (qi + 1) * NQ] = (
            r.transpose(1, 0, 2).reshape(C, NQ)
        )
    return out.reshape(B, C, HW, HW)


def _run(inputs, trace=False, trace_kwargs=None):
    nc = _get_nc()
    in_maps = _make_in_maps(**inputs)
    res = run_bass_kernel_spmd(
        nc, in_maps, list(range(NCORES)), trace=trace,
        **(trace_kwargs or {}),
    )
    return res


def kernel(**inputs):
    res = _run(inputs)
    return _assemble(res.results)


# revision 9
# speedup vs baseline: 1.5087x; 1.2007x over previous
"""AttnBlock (B=2, C=512, H=W=64) on 8 TRN2 NeuronCores.

Sharding: core c handles batch b=c//4 and query-quarter q=c%4 (1024 of 4096
query positions). Keys/values are computed redundantly per core from the
full batch image (group-norm needs all of it anyway). The key axis is
host-permuted per core so the core's query quarter occupies columns 0:1024
of its buffer — softmax/attention are permutation-invariant over keys, so
the same SPMD program works on every core with no dynamic indexing.

All matmuls run in fp8 e4m3 DoubleRow mode (two 128-deep contraction
subtiles per instruction, 2x PE rate). x is shipped bf16; group-norm
statistics run on bf16 via bn_stats. Softmax runs without max-subtraction
(logits ~N(0,1)); exp writes fp8 probability pair-tiles that feed the PE
directly: an all-ones stationary produces Z replicated across partitions
(so 1/Z is one full-width reciprocal), and U = V P^T is accumulated
directly in [c, i] layout so no transposes are needed — U is normalized
by 1/Z during the PSUM->fp8 cast and fed straight to the output
projection. The residual path stays exact fp32.
"""

import numpy as np
import ml_dtypes

import concourse.bass as bass
import concourse.tile as tile
from concourse import bacc, mybir
from concourse.bass_utils import run_bass_kernel_spmd

F32 = mybir.dt.float32
BF16 = mybir.dt.bfloat16
F8 = mybir.dt.float8e4
DR = mybir.MatmulPerfMode.DoubleRow
E4 = ml_dtypes.float8_e4m3fn

P = 128          # partitions
CT = 4           # channel tiles (C = 512 = 4*128)
C = 512
N = 4096         # H*W
NS = 8           # 512-wide column slices of N
NJT = 32         # 128-wide key tiles
NPAIR = 16       # key-tile pairs (DoubleRow contraction)
NQ = 1024        # query columns per core
B = 2
HW = 64
NGROUPS = 32
GSIZE = C // NGROUPS  # 16 channels per group
EPS = 1e-5
SCL = float(C) ** -0.5
NCORES = 8

_cached = {}


def _ct_layout(v):
    """[C] -> [P, CT] with channel c at [c % 128, c // 128]."""
    return np.ascontiguousarray(v.reshape(CT, P).T, dtype=np.float32)


def _cmaj(a2d, ncols, dtype=np.float32):
    """[C, ncols] -> [P, CT, ncols]."""
    return np.ascontiguousarray(
        a2d.reshape(CT, P, ncols).transpose(1, 0, 2).astype(dtype)
    )


def _build_program():
    nc = bacc.Bacc("TRN2", target_bir_lowering=False, debug=False)

    X_d = nc.declare_dram_parameter("xin", [P, CT, N], BF16, isOutput=False)
    XQ_d = nc.declare_dram_parameter("xq", [P, CT, NQ], F32, isOutput=False)
    WQ_d = nc.declare_dram_parameter("wqt", [P, CT, C], F8, isOutput=False)
    WK_d = nc.declare_dram_parameter("wkt", [P, CT, C], F8, isOutput=False)
    WV_d = nc.declare_dram_parameter("wvt", [P, CT, C], F8, isOutput=False)
    WP_d = nc.declare_dram_parameter("wpt", [P, CT, C], F8, isOutput=False)
    BQ_d = nc.declare_dram_parameter("bq2", [P, CT], F32, isOutput=False)
    BK_d = nc.declare_dram_parameter("bk2", [P, CT], F32, isOutput=False)
    BPE_d = nc.declare_dram_parameter("bpe", [P, CT], F32, isOutput=False)
    GAM_d = nc.declare_dram_parameter("gam", [P, CT], F32, isOutput=False)
    BET_d = nc.declare_dram_parameter("bet", [P, CT], F32, isOutput=False)
    G_d = nc.declare_dram_parameter("gmat", [P, CT, NGROUPS], F32, isOutput=False)
    E_d = nc.declare_dram_parameter("emat", [NGROUPS, CT, P], F32, isOutput=False)
    ONE_d = nc.declare_dram_parameter("ones8", [P, 2, P], F8, isOutput=False)
    OUT_d = nc.declare_dram_parameter("out", [P, CT, NQ], F32, isOutput=True)

    with tile.TileContext(nc) as tc:
        with (
            tc.tile_pool(name="big", bufs=1) as big,
            tc.tile_pool(name="consts", bufs=1) as consts,
            tc.tile_pool(name="stat", bufs=1) as stat,
        ):
            XB = big.tile([P, CT, N], BF16)
            H8 = big.tile([P, CT, N], F8)
            K8 = big.tile([P, CT, N], F8)
            V8 = big.tile([P, NJT, C], F8)
            Q8 = big.tile([P, CT, NQ], F8)
            U8 = big.tile([P, CT, NQ], F8)
            XQT = big.tile([P, CT, NQ], F32)

            wq = consts.tile([P, CT, C], F8)
            wk = consts.tile([P, CT, C], F8)
            wv = consts.tile([P, CT, C], F8)
            wp = consts.tile([P, CT, C], F8)
            ones8 = consts.tile([P, 2, P], F8)
            bpe_sb = consts.tile([P, CT], F32)
            bq_sb = consts.tile([P, CT], F32)
            bk_sb = consts.tile([P, CT], F32)
            gam_sb = consts.tile([P, CT], F32)
            bet_sb = consts.tile([P, CT], F32)
            gmat = consts.tile([P, CT, NGROUPS], F32)
            emat = consts.tile([NGROUPS, CT, P], F32)
            # logit shift: exp(SCL*s - 2) keeps p under fp8e4m3 max (448)
            # while staying softmax-invariant (cancels in U/Z)
            neg2 = consts.tile([P, 1], F32)
            nc.vector.memset(neg2, -2.0)

            # input DMAs issue from three queues in parallel: X slices
            # (stats critical path) on sync, stats consts on gpsimd,
            # weights/biases (needed from phase 2 on) on scalar
            for s in range(NS):
                sl = slice(s * 512, (s + 1) * 512)
                nc.sync.dma_start(out=XB[:, :, sl], in_=X_d[:, :, sl])
            nc.sync.dma_start(out=XQT, in_=XQ_d[:])
            nc.gpsimd.dma_start(out=gmat, in_=G_d[:])
            nc.gpsimd.dma_start(out=emat, in_=E_d[:])
            nc.gpsimd.dma_start(out=gam_sb, in_=GAM_d[:])
            nc.gpsimd.dma_start(out=bet_sb, in_=BET_d[:])
            nc.gpsimd.dma_start(out=ones8, in_=ONE_d[:])
            nc.scalar.dma_start(out=wq, in_=WQ_d[:])
            nc.scalar.dma_start(out=wk, in_=WK_d[:])
            nc.scalar.dma_start(out=wv, in_=WV_d[:])
            nc.scalar.dma_start(out=bq_sb, in_=BQ_d[:])
            nc.scalar.dma_start(out=bk_sb, in_=BK_d[:])
            nc.scalar.dma_start(out=wp, in_=WP_d[:])
            nc.scalar.dma_start(out=bpe_sb, in_=BPE_d[:])

            # ---------------- Phase 1: group-norm statistics ----------------
            bnst = stat.tile([P, CT, NS, 6], F32)
            for s in range(NS):
                for t in range(CT):
                    nc.vector.bn_stats(
                        out=bnst[:, t, s, :],
                        in_=XB[:, t, s * 512 : (s + 1) * 512],
                    )
            mex = stat.tile([P, CT, 2], F32)
            for t in range(CT):
                nc.vector.bn_aggr(out=mex[:, t, :], in_=bnst[:, t, :, :])
            # mexp[...,0] = mean, mexp[...,1] = E[x^2] = var + mean^2
            mexp = stat.tile([P, CT, 2], F32)
            nc.vector.tensor_copy(out=mexp[:, :, 0], in_=mex[:, :, 0])
            nc.vector.tensor_tensor(
                out=mexp[:, :, 1], in0=mex[:, :, 0], in1=mex[:, :, 0],
                op=mybir.AluOpType.mult,
            )
            nc.vector.tensor_add(
                out=mexp[:, :, 1], in0=mexp[:, :, 1], in1=mex[:, :, 1]
            )

            scale_c = stat.tile([P, CT], F32)
            shift_c = stat.tile([P, CT], F32)
            with tc.tile_pool(name="psum_p1", bufs=1, space="PSUM") as p1:
                gs_ps = p1.tile([NGROUPS, 2], F32, tag="gs")
                for t in range(CT):
                    nc.tensor.matmul(
                        gs_ps, gmat[:, t, :], mexp[:, t, :],
                        start=(t == 0), stop=(t == CT - 1),
                    )
                gsb = stat.tile([NGROUPS, 2], F32)
                nc.vector.tensor_copy(out=gsb, in_=gs_ps)
                gmr = stat.tile([NGROUPS, 2], F32)
                gtmp = stat.tile([NGROUPS, 2], F32)
                nc.vector.tensor_scalar_mul(
                    out=gmr[:, 0:1], in0=gsb[:, 0:1], scalar1=1.0 / GSIZE
                )
                nc.vector.tensor_scalar_mul(
                    out=gtmp[:, 0:1], in0=gsb[:, 1:2], scalar1=1.0 / GSIZE
                )
                nc.vector.tensor_tensor(
                    out=gtmp[:, 1:2], in0=gmr[:, 0:1], in1=gmr[:, 0:1],
                    op=mybir.AluOpType.mult,
                )
                nc.vector.tensor_sub(
                    out=gtmp[:, 0:1], in0=gtmp[:, 0:1], in1=gtmp[:, 1:2]
                )
                eps_sb = stat.tile([NGROUPS, 1], F32)
                nc.vector.memset(eps_sb, EPS)
                nc.scalar.activation(
                    out=gtmp[:, 0:1], in_=gtmp[:, 0:1],
                    func=mybir.ActivationFunctionType.Sqrt, bias=eps_sb,
                )
                nc.vector.reciprocal(out=gmr[:, 1:2], in_=gtmp[:, 0:1])
                mc = stat.tile([P, CT, 2], F32)
                for t in range(CT):
                    ms_ps = p1.tile([P, 2], F32, tag="ms")
                    nc.tensor.matmul(ms_ps, emat[:, t, :], gmr, start=True, stop=True)
                    nc.vector.tensor_copy(out=mc[:, t, :], in_=ms_ps)
                nc.vector.tensor_tensor(
                    out=scale_c, in0=mc[:, :, 1], in1=gam_sb, op=mybir.AluOpType.mult
                )
                nc.vector.tensor_tensor(
                    out=shift_c, in0=mc[:, :, 0], in1=scale_c, op=mybir.AluOpType.mult
                )
                nc.vector.tensor_sub(out=shift_c, in0=bet_sb, in1=shift_c)

            # ---------------- Phase 2: normalize + q/k/v projections --------
            def norm_slice(s, eng=None):
                sl = slice(s * 512, (s + 1) * 512)
                for t in range(CT):
                    (eng or nc.gpsimd).tensor_scalar(
                        out=H8[:, t, sl],
                        in0=XB[:, t, sl],
                        scalar1=scale_c[:, t : t + 1],
                        scalar2=shift_c[:, t : t + 1],
                        op0=mybir.AluOpType.mult,
                        op1=mybir.AluOpType.add,
                    )

            with tc.tile_pool(name="psum2", bufs=1, space="PSUM") as psum2:
                norm_slice(0, eng=nc.vector)
                for s in range(NS):
                    if s + 1 < NS:
                        norm_slice(s + 1)
                    sl = slice(s * 512, (s + 1) * 512)
                    if s < 2:
                        for ct in range(CT):
                            qp = psum2.tile([P, 512], F32, tag="acc", bufs=3)
                            for m in range(2):
                                nc.tensor.matmul(
                                    qp,
                                    wq[:, 2 * m : 2 * m + 2, ct * P : (ct + 1) * P],
                                    H8[:, 2 * m : 2 * m + 2, sl],
                                    start=(m == 0), stop=(m == 1), perf_mode=DR,
                                )
                            nc.vector.tensor_scalar_add(
                                out=Q8[:, ct, sl],
                                in0=qp,
                                scalar1=bq_sb[:, ct : ct + 1],
                            )
                    for jt in range(CT):
                        vp = psum2.tile([P, 512], F32, tag="acc", bufs=3)
                        jcol = slice(s * 512 + jt * P, s * 512 + (jt + 1) * P)
                        for m in range(2):
                            nc.tensor.matmul(
                                vp,
                                H8[:, 2 * m : 2 * m + 2, jcol],
                                wv[:, 2 * m : 2 * m + 2, :],
                                start=(m == 0), stop=(m == 1), perf_mode=DR,
                            )
                        # v scaled by 1/4 so unnormalized U stays inside
                        # fp8e4m3 range; ones8=0.25 scales Z to match
                        nc.vector.tensor_scalar_mul(
                            out=V8[:, s * 4 + jt, :], in0=vp, scalar1=0.25
                        )
                    for ct in range(CT):
                        kp = psum2.tile([P, 512], F32, tag="acc", bufs=3)
                        for m in range(2):
                            nc.tensor.matmul(
                                kp,
                                wk[:, 2 * m : 2 * m + 2, ct * P : (ct + 1) * P],
                                H8[:, 2 * m : 2 * m + 2, sl],
                                start=(m == 0), stop=(m == 1), perf_mode=DR,
                            )
                        nc.scalar.activation(
                            out=K8[:, ct, sl], in_=kp,
                            func=mybir.ActivationFunctionType.Identity,
                            bias=bk_sb[:, ct : ct + 1],
                        )

            # ---------------- Phase 3: attention --------------------------
            with (
                tc.tile_pool(name="psum3", bufs=1, space="PSUM") as psum3,
                tc.tile_pool(name="pwork", bufs=1) as pwork,
            ):
                proj_jobs = []

                def pop_proj():
                    if proj_jobs:
                        proj_jobs.pop(0)()

                def st_exp(isl, jt, pt):
                    """S^T matmuls + exp into pair-tile half jt%2."""
                    s_ps = psum3.tile([P, 512], F32, tag="s", bufs=2)
                    isl_sl = slice(isl * 512, (isl + 1) * 512)
                    jb = slice(jt * P, (jt + 1) * P)
                    for m in range(2):
                        nc.tensor.matmul(
                            s_ps,
                            K8[:, 2 * m : 2 * m + 2, jb],
                            Q8[:, 2 * m : 2 * m + 2, isl_sl],
                            start=(m == 0), stop=(m == 1), perf_mode=DR,
                        )
                    nc.scalar.activation(
                        out=pt[:, jt % 2, :], in_=s_ps,
                        func=mybir.ActivationFunctionType.Exp, scale=SCL,
                        bias=neg2,
                    )

                def proj_group(h, ct, zb):
                    """Projection on unnormalized U8, then x(1/Z) + bias +
                    residual at emit. Requires U8 cols of i-slice h final."""
                    sl = slice(h * 512, (h + 1) * 512)
                    pr = psum3.tile([P, 512], F32, tag="s", bufs=2)
                    for m in range(2):
                        nc.tensor.matmul(
                            pr,
                            wp[:, 2 * m : 2 * m + 2, ct * P : (ct + 1) * P],
                            U8[:, 2 * m : 2 * m + 2, sl],
                            start=(m == 0), stop=(m == 1), perf_mode=DR,
                        )
                    prz = pwork.tile([P, 512], F32, tag="prz", bufs=3)
                    nc.vector.tensor_tensor(
                        out=prz, in0=pr, in1=zb, op=mybir.AluOpType.mult
                    )
                    ost = pwork.tile([P, 512], F32, tag="ost", bufs=3)
                    nc.vector.scalar_tensor_tensor(
                        out=ost, in0=prz, scalar=bpe_sb[:, ct : ct + 1],
                        in1=XQT[:, ct, sl], op0=mybir.AluOpType.add,
                        op1=mybir.AluOpType.add,
                    )
                    nc.sync.dma_start(out=OUT_d[:, ct, sl], in_=ost)

                for isl in range(2):
                    u_ps = [
                        psum3.tile([P, 512], F32, tag=f"u{cc}", bufs=1,
                                   name=f"u{cc}")
                        for cc in range(CT)
                    ]
                    z_ps = psum3.tile([P, 512], F32, tag="z", bufs=2, name="z")
                    for pair in range(NPAIR):
                        pt = pwork.tile([P, 2, 512], F8, tag="pt", bufs=3)
                        st_exp(isl, 2 * pair, pt)
                        st_exp(isl, 2 * pair + 1, pt)
                        nc.tensor.matmul(
                            z_ps, ones8, pt,
                            start=(pair == 0), stop=(pair == NPAIR - 1),
                            perf_mode=DR,
                        )
                        for cc in range(CT):
                            nc.tensor.matmul(
                                u_ps[cc],
                                V8[:, 2 * pair : 2 * pair + 2,
                                   cc * P : (cc + 1) * P],
                                pt,
                                start=(pair == 0), stop=(pair == NPAIR - 1),
                                perf_mode=DR,
                            )
                        # interleave i-slice 0's output projection into
                        # i-slice 1's key loop so the PE never stalls
                        if isl == 1 and pair >= 2 and (pair - 2) % 3 == 0:
                            pop_proj()
                    isl_sl = slice(isl * 512, (isl + 1) * 512)
                    for cc in range(CT):
                        nc.vector.tensor_copy(
                            out=U8[:, cc, isl_sl], in_=u_ps[cc]
                        )
                    zb = pwork.tile([P, 512], F32, tag="zb", bufs=2)
                    nc.vector.reciprocal(out=zb, in_=z_ps)
                    for ct in range(CT):
                        proj_jobs.append(
                            lambda h=isl, ct=ct, zb=zb: proj_group(h, ct, zb)
                        )

                while proj_jobs:
                    pop_proj()

    nc.compile()
    return nc


def _get_nc():
    if "nc" not in _cached:
        _cached["nc"] = _build_program()
    return _cached["nc"]


def _make_in_maps(x, norm_gamma, norm_beta, wq, bq, wk, bk, wv, bv, wp, bp):
    gm = np.zeros((P, CT, NGROUPS), np.float32)
    em = np.zeros((NGROUPS, CT, P), np.float32)
    for t in range(CT):
        for p in range(P):
            g = (t * P + p) // GSIZE
            gm[p, t, g] = 1.0
            em[g, t, p] = 1.0

    common = {
        "wqt": _cmaj(np.asarray(wq).T, C, E4),
        "wkt": _cmaj(np.asarray(wk).T, C, E4),
        "wvt": _cmaj(np.asarray(wv).T, C, E4),
        "wpt": _cmaj(np.asarray(wp).T, C, E4),
        "bq2": _ct_layout(np.asarray(bq)),
        "bk2": _ct_layout(np.asarray(bk)),
        "bpe": _ct_layout(np.asarray(bp) + np.asarray(wp) @ np.asarray(bv)),
        "gam": _ct_layout(np.asarray(norm_gamma)),
        "bet": _ct_layout(np.asarray(norm_beta)),
        "gmat": gm,
        "emat": em,
        "ones8": np.full((P, 2, P), 0.25, dtype=E4),
    }

    in_maps = []
    for c in range(NCORES):
        b, qi = c // 4, c % 4
        xb = np.asarray(x[b], dtype=np.float32).reshape(C, N)
        xp = np.concatenate([xb[:, qi * NQ :], xb[:, : qi * NQ]], axis=1)
        m = dict(common)
        m["xin"] = _cmaj(xp, N, ml_dtypes.bfloat16)
        m["xq"] = _cmaj(xb[:, qi * NQ : (qi + 1) * NQ], NQ)
        in_maps.append(m)
    return in_maps


def _assemble(results):
    out = np.empty((B, C, N), np.float32)
    for c in range(NCORES):
        b, qi = c // 4, c % 4
        r = results[c]["out"]  # [P, CT, NQ]
        out[b, :, qi * NQ : (qi + 1) * NQ] = (
            r.transpose(1, 0, 2).reshape(C, NQ)
        )
    return out.reshape(B, C, HW, HW)


def _run(inputs, trace=False, trace_kwargs=None):
    nc = _get_nc()
    in_maps = _make_in_maps(**inputs)
    res = run_bass_kernel_spmd(
        nc, in_maps, list(range(NCORES)), trace=trace,
        **(trace_kwargs or {}),
    )
    return res


def kernel(**inputs):
    res = _run(inputs)
    return _assemble(res.results)
